# revision 14
# baseline (speedup 1.0000x reference)
"""Trainium2 Bass kernel for causal multi-head attention (B=2, S=2048, E=1024, H=16).

Sharding: 8 cores = 2 batches x 4 head-groups (4 heads each).
Each core computes its batch's QKV for its 4 heads, causal attention, and a
partial output projection; host sums the 4 group partials per batch + b_out.

All big matmuls run in float32r (TF32-like, 1 cycle/row at N>=256).
"""
import sys

sys.path.insert(0, "/opt/trn_rl_repo")

from contextlib import ExitStack

import numpy as np

import concourse.bass as bass
import concourse.tile as tile
from concourse import bacc, mybir
from concourse.bass_utils import run_bass_kernel_spmd

dt = mybir.dt

B, S, E, H = 2, 2048, 1024, 16
HD = 64                     # head dim
HPC = 4                     # heads per core
NC = 8                      # cores
KE = E // 128               # 8 contraction k-tiles for projections
NT = S // 128               # 16 token tiles
NCH = S // 512              # 4 token chunks
FQK = 512                   # q+k features per core (4 heads * 64 * 2)
FV = 256                    # v features per core

# engine used for fp32 -> fp32r rounding copies of DMA'd inputs
ROUND_ENGINE = "gpsimd"


def _build_program():
    nc = bacc.Bacc("TRN2", target_bir_lowering=False, debug=False, num_devices=NC)

    xT_d = nc.dram_tensor("xT", [E, S], dt.float32, kind="ExternalInput")
    wqkT_d = nc.dram_tensor("wqkT", [E, FQK], dt.float32, kind="ExternalInput")
    wvT_d = nc.dram_tensor("wvT", [E, FV], dt.float32, kind="ExternalInput")
    bqk_d = nc.dram_tensor("bqk", [FQK], dt.float32, kind="ExternalInput")
    bv_d = nc.dram_tensor("bv", [FV], dt.float32, kind="ExternalInput")
    wo_d = nc.dram_tensor("wo", [FV, E], dt.float32, kind="ExternalInput")
    mask_d = nc.dram_tensor("trimask", [128, 128], dt.float32, kind="ExternalInput")
    y_d = nc.dram_tensor("y", [S, E], dt.float32, kind="ExternalOutput")

    with TileKernel(nc) as tk:
        tk.build(xT_d, wqkT_d, wvT_d, bqk_d, bv_d, wo_d, mask_d, y_d)
    nc.compile()
    return nc


class TileKernel:
    def __init__(self, nc):
        self.nc = nc
        self.ctx = ExitStack()
        self.tc_cm = tile.TileContext(nc)

    def __enter__(self):
        self.tc = self.tc_cm.__enter__()
        return self

    def __exit__(self, *a):
        self.ctx.close()
        return self.tc_cm.__exit__(*a)

    _round_i = 0

    def round_copy(self, out, in_):
        nc = self.nc
        nc.vector.tensor_copy(out, in_)

    def build(self, xT_d, wqkT_d, wvT_d, bqk_d, bv_d, wo_d, mask_d, y_d):
        nc, tc, ctx = self.nc, self.tc, self.ctx
        pool = lambda name, bufs, **kw: ctx.enter_context(
            tc.tile_pool(name=name, bufs=bufs, **kw)
        )

        const_p = pool("const", 1)
        wstage_p = pool("wstage", 2)
        xs_p = pool("xs", 1)
        xr_p = pool("xr", 2)
        qkt_p = pool("qkt", 1)
        vones_p = pool("vones", 1)
        attn_p = pool("attn", 3)
        pair_p = pool("pair", 1)
        small_p = pool("small", 2)
        y_p = pool("y", 3)
        # PSUM: ps (2 banks x 2 bufs) + po (1 bank x 2 bufs x 2 tags) = 8
        ps_p = pool("ps", 2, space="PSUM")
        po_p = pool("po", 2, space="PSUM")
        p1_p = ps_p  # qkv/outproj psums share the ps slots

        # ---- constants / weights ----
        bqk_sb = const_p.tile([128, 4], dt.float32, tag="bqk")
        nc.sync.dma_start(bqk_sb[:], bqk_d[:].rearrange("(f p) -> p f", p=128))
        bv_sb = const_p.tile([128, 2], dt.float32, tag="bv")
        nc.sync.dma_start(bv_sb[:], bv_d[:].rearrange("(f p) -> p f", p=128))
        ones_sb = const_p.tile([128, 1, 1], dt.float32, tag="ones")
        nc.vector.memset(ones_sb[:], 1.0)
        mask_sb = const_p.tile([128, 128], dt.float32, tag="mask")
        nc.sync.dma_start(mask_sb[:], mask_d[:])

        wqk_r = []
        for ke in range(KE):
            st = wstage_p.tile([128, FQK], dt.float32, tag="wst")
            nc.sync.dma_start(st[:], wqkT_d[128 * ke : 128 * (ke + 1), :])
            wr = const_p.tile([128, FQK], dt.float32r, tag=f"wqk{ke}")
            self.round_copy(wr[:], st[:])
            wqk_r.append(wr)
        wv_r = []
        for ke in range(KE):
            st = wstage_p.tile([128, FV], dt.float32, tag="wst2")
            nc.sync.dma_start(st[:], wvT_d[128 * ke : 128 * (ke + 1), :])
            wr = const_p.tile([128, FV], dt.float32r, tag=f"wv{ke}")
            self.round_copy(wr[:], st[:])
            wv_r.append(wr)
        wo_r = []
        for kt in range(2):
            st = wstage_p.tile([128, E], dt.float32, tag="wst3")
            nc.sync.dma_start(st[:], wo_d[128 * kt : 128 * (kt + 1), :])
            wr = const_p.tile([128, E], dt.float32r, tag=f"wo{kt}")
            self.round_copy(wr[:], st[:])
            wo_r.append(wr)

        # ---- persistent activations ----
        # qkt tiles: 0: q heads 0,1 | 1: q heads 2,3 | 2: k heads 0,1 | 3: k heads 2,3
        qkt = [qkt_p.tile([128, S], dt.float32r, tag=f"qkt{f}", name=f"qkt{f}") for f in range(4)]
        # vones[t]: [v h0 |1| v h1 |1| v h2 |1| v h3 |1] for token tile t
        vones = [vones_p.tile([128, 4 * 65], dt.float32r, tag=f"v{t}", name=f"v{t}") for t in range(NT)]
        # pair tiles: final normalized attn output, [head dims x 2, S]
        pairt = [pair_p.tile([128, S], dt.float32r, tag=f"pair{hp}", name=f"pair{hp}") for hp in range(2)]

        for c in range(NCH):
            with nc.named_scope(f"qkv{c}"):
                self.qkv_chunk(c, xT_d, wqk_r, wv_r, bqk_sb, ones_sb,
                               xs_p, xr_p, p1_p, qkt, vones)
            for hp in range(2):
                with nc.named_scope(f"attn{c}_{hp}"):
                    self.attention(hp, c, qkt, vones, ps_p, po_p, attn_p,
                                   small_p, pairt, bv_sb, mask_sb)
            with nc.named_scope(f"oproj{c}"):
                self.out_proj(c, pairt, wo_r, p1_p, y_p, y_d)

    # ------------------------------------------------------------------
    def qkv_chunk(self, c, xT_d, wqk_r, wv_r, bqk_sb, ones_sb,
                  xs_p, xr_p, p1_p, qkt, vones):
        nc = self.nc
        cs = slice(512 * c, 512 * (c + 1))
        xr = []
        for ke in range(KE):
            xs = xs_p.tile([128, 512], dt.float32, tag=f"xs{ke}")
            nc.sync.dma_start(xs[:], xT_d[128 * ke : 128 * (ke + 1), cs])
            xt = xr_p.tile([128, 512], dt.float32r, tag=f"xr{ke}")
            self.round_copy(xt[:], xs[:])
            xr.append(xt)
        # q/k projection: qkT[f-tile, chunk] = wqkT.T @ xT
        for f in range(4):
            pq = p1_p.tile([128, 1024], dt.float32, tag="ps", name="pq")
            for ke in range(KE):
                nc.tensor.matmul(
                    pq[:, 0:512], wqk_r[ke][:, 128 * f : 128 * (f + 1)], xr[ke][:],
                    start=(ke == 0), stop=(ke == KE - 1),
                )
            nc.vector.tensor_scalar_add(qkt[f][:, cs], pq[:, 0:512], bqk_sb[:, f : f + 1])
        # v projection, natural layout [tok, vfeat] (no bias: folded post-softmax)
        for t4 in range(4):
            t = 4 * c + t4
            pv = p1_p.tile([128, 1024], dt.float32, tag="ps", name="pv")
            for ke in range(KE):
                nc.tensor.matmul(
                    pv[:, 0:FV],
                    xr[ke][:, 128 * t4 : 128 * (t4 + 1)], wv_r[ke][:],
                    start=(ke == 0), stop=(ke == KE - 1),
                )
            vt = vones[t]
            v3 = vt[:].rearrange("p (g d) -> p g d", d=65)
            nc.vector.tensor_copy(
                v3[:, :, 0:64],
                pv[:, 0:FV].rearrange("p (g d) -> p g d", d=64),
            )
            nc.vector.tensor_copy(v3[:, :, 64:65], ones_sb[:].to_broadcast((128, 4, 1)))

    # ------------------------------------------------------------------
    def attention(self, hp, c, qkt, vones, ps_p, po_p, attn_p, small_p,
                  pairt, bv_sb, mask_sb):
        nc = self.nc
        nj = 4 * c + 4
        po = [po_p.tile([65, 512], dt.float32, tag=f"po{i}", name=f"po{i}") for i in range(2)]

        def emit_pv(j, off, at):
            for h_idx in range(2):
                slot = 2 * hp + h_idx
                nc.tensor.matmul(
                    po[h_idx][:, off:512],
                    vones[j][:, 65 * slot : 65 * slot + 65],
                    at[:, 512 * h_idx + off : 512 * (h_idx + 1)],
                    start=(j == 0), stop=(j == nj - 1),
                    skip_group_check=True,
                )

        prev = None
        for j in range(nj):
            ps = ps_p.tile([128, 1024], dt.float32, tag="ps", name="ps")
            at = attn_p.tile([128, 1024], dt.float32r, tag="attn", name="at")
            m = j - 4 * c
            off = 128 * m if 1 <= m <= 3 else 0
            off_mm = off if m in (1, 2) else 0
            for h_idx in range(2):
                r0 = 64 * h_idx
                nc.tensor.matmul(
                    ps[:, 512 * h_idx + off_mm : 512 * (h_idx + 1)],
                    qkt[2 + hp][r0 : r0 + 64, 128 * j : 128 * (j + 1)],
                    qkt[hp][r0 : r0 + 64, 512 * c + off_mm : 512 * (c + 1)],
                    start=True, stop=True,
                )
            if m >= 0:
                for h_idx in range(2):
                    lo = 512 * h_idx + 128 * m
                    nc.vector.tensor_add(ps[:, lo : lo + 128], ps[:, lo : lo + 128], mask_sb[:])
            if off == 0:
                runs = [(0, 1024)]
            else:
                runs = [(off, 512), (512 + off, 1024)]
            for lo, hi in runs:
                nc.scalar.activation(at[:, lo:hi], ps[:, lo:hi], mybir.ActivationFunctionType.Exp)
            if prev is not None:
                emit_pv(*prev)
            prev = (j, off, at)
        emit_pv(*prev)
        # rollout: normalize by denominator (row 64) and add v bias
        Ln, Exp = mybir.ActivationFunctionType.Ln, mybir.ActivationFunctionType.Exp
        rpa = small_p.tile([1, 1, 512], dt.float32, tag="recipa")
        rpb = small_p.tile([1, 1, 512], dt.float32, tag="recipb")
        nc.scalar.activation(rpa[0:1, 0, :], po[0][64:65, :], Ln)
        nc.scalar.activation(rpa[0:1, 0, :], rpa[0:1, 0, :], Exp, scale=-1.0)
        nc.scalar.activation(rpb[0:1, 0, :], po[1][64:65, :], Ln)
        nc.scalar.activation(rpb[0:1, 0, :], rpb[0:1, 0, :], Exp, scale=-1.0)
        bc = small_p.tile([128, 512], dt.float32, tag="bcast")
        nc.gpsimd.dma_start(bc[0:64, :], rpa[:].to_broadcast((1, 64, 512)))
        nc.gpsimd.dma_start(bc[64:128, :], rpb[:].to_broadcast((1, 64, 512)))
        tmp = small_p.tile([128, 512], dt.float32, tag="tmp")
        nc.vector.tensor_mul(tmp[0:64, :], po[0][0:64, :], bc[0:64, :])
        nc.vector.tensor_mul(tmp[64:128, :], po[1][0:64, :], bc[64:128, :])
        nc.vector.tensor_scalar_add(
            pairt[hp][:, 512 * c : 512 * (c + 1)], tmp[:], bv_sb[:, hp : hp + 1]
        )

    # ------------------------------------------------------------------
    def out_proj(self, c, pairt, wo_r, p1_p, y_p, y_d):
        nc = self.nc
        for t4 in range(4):
            t = 4 * c + t4
            ysb = y_p.tile([128, E], dt.float32, tag="y")
            for o in range(2):
                py = p1_p.tile([128, 1024], dt.float32, tag="ps", name="py")
                for kt in range(2):
                    nc.tensor.matmul(
                        py[:, 0:512],
                        pairt[kt][:, 128 * t : 128 * (t + 1)],
                        wo_r[kt][:, 512 * o : 512 * (o + 1)],
                        start=(kt == 0), stop=(kt == 1),
                    )
                if o == 0:
                    nc.vector.tensor_copy(ysb[:, 0:512], py[:, 0:512])
                else:
                    nc.scalar.activation(
                        ysb[:, 512:1024], py[:, 0:512], mybir.ActivationFunctionType.Copy
                    )
            nc.gpsimd.dma_start(y_d[128 * t : 128 * (t + 1), :], ysb[:])


# ----------------------------------------------------------------------
_PROGRAM = None


def _get_program():
    global _PROGRAM
    if _PROGRAM is None:
        _PROGRAM = _build_program()
    return _PROGRAM


def _make_in_maps(inputs, W_in, b_in, W_out, b_out):
    in_maps = []
    scale = 1.0 / np.sqrt(np.float32(HD))
    kr = np.arange(128)[:, None]
    qc = np.arange(128)[None, :]
    trimask = np.where(qc >= kr, 0.0, -1e30).astype(np.float32)
    for core in range(NC):
        b, g = divmod(core, 4)
        r = slice(256 * g, 256 * (g + 1))
        wq = W_in[0:E][r] * scale
        wk = W_in[E : 2 * E][r]
        wv = W_in[2 * E : 3 * E][r]
        xT = np.ascontiguousarray(inputs[b].T.astype(np.float32))
        wqkT = np.ascontiguousarray(np.concatenate([wq, wk], axis=0).T)
        wvT = np.ascontiguousarray(wv.T)
        bqk = np.concatenate([b_in[0:E][r] * scale, b_in[E : 2 * E][r]])
        bv = np.ascontiguousarray(b_in[2 * E : 3 * E][r])
        wo = np.ascontiguousarray(W_out[:, r].T)
        in_maps.append(
            {
                "xT": xT,
                "wqkT": wqkT.astype(np.float32),
                "wvT": wvT.astype(np.float32),
                "bqk": bqk.astype(np.float32),
                "bv": bv.astype(np.float32),
                "wo": wo.astype(np.float32),
                "trimask": trimask,
            }
        )
    return in_maps


def run_spmd(inputs, W_in, b_in, W_out, b_out, trace=False, **kw):
    nc = _get_program()
    in_maps = _make_in_maps(inputs, W_in, b_in, W_out, b_out)
    bkr = run_bass_kernel_spmd(nc, in_maps, list(range(NC)), trace=trace, **kw)
    parts = [bkr.results[i]["y"] for i in range(NC)]
    out = np.stack(
        [
            parts[0] + parts[1] + parts[2] + parts[3],
            parts[4] + parts[5] + parts[6] + parts[7],
        ]
    )
    out = out + b_out[None, None, :]
    return out.astype(np.float32), bkr


def kernel(inputs, W_in, b_in, W_out, b_out):
    out, _ = run_spmd(
        np.asarray(inputs, dtype=np.float32),
        np.asarray(W_in, dtype=np.float32),
        np.asarray(b_in, dtype=np.float32),
        np.asarray(W_out, dtype=np.float32),
        np.asarray(b_out, dtype=np.float32),
    )
    return out


# revision 17
# speedup vs baseline: 1.0402x; 1.0402x over previous
"""Trainium2 Bass kernel for causal multi-head attention (B=2, S=2048, E=1024, H=16).

Sharding: 8 cores = 2 batches x 4 head-groups (4 heads each).
Each core computes its batch's QKV for its 4 heads, causal attention, and a
partial output projection; host sums the 4 group partials per batch + b_out.

All big matmuls run in float32r (TF32-like, 1 cycle/row at N>=256).
"""
import sys

sys.path.insert(0, "/opt/trn_rl_repo")

from contextlib import ExitStack

import numpy as np

import concourse.bass as bass
import concourse.tile as tile
from concourse import bacc, mybir
from concourse.bass_utils import run_bass_kernel_spmd

dt = mybir.dt

B, S, E, H = 2, 2048, 1024, 16
HD = 64                     # head dim
HPC = 4                     # heads per core
NC = 8                      # cores
KE = E // 128               # 8 contraction k-tiles for projections
NT = S // 128               # 16 token tiles
NCH = S // 512              # 4 token chunks
FQK = 512                   # q+k features per core (4 heads * 64 * 2)
FV = 256                    # v features per core

# engine used for fp32 -> fp32r rounding copies of DMA'd inputs
ROUND_ENGINE = "gpsimd"


def _build_program():
    nc = bacc.Bacc("TRN2", target_bir_lowering=False, debug=False, num_devices=NC)

    xT_d = nc.dram_tensor("xT", [E, S], dt.float32, kind="ExternalInput")
    wqkT_d = nc.dram_tensor("wqkT", [E, FQK], dt.float32, kind="ExternalInput")
    wvT_d = nc.dram_tensor("wvT", [E, FV], dt.float32, kind="ExternalInput")
    bqk_d = nc.dram_tensor("bqk", [FQK], dt.float32, kind="ExternalInput")
    bv_d = nc.dram_tensor("bv", [FV], dt.float32, kind="ExternalInput")
    wo_d = nc.dram_tensor("wo", [FV, E], dt.float32, kind="ExternalInput")
    mask_d = nc.dram_tensor("trimask", [128, 128], dt.float32, kind="ExternalInput")
    y_d = nc.dram_tensor("y", [S, E], dt.float32, kind="ExternalOutput")

    with TileKernel(nc) as tk:
        tk.build(xT_d, wqkT_d, wvT_d, bqk_d, bv_d, wo_d, mask_d, y_d)
    nc.compile()
    return nc


class TileKernel:
    def __init__(self, nc):
        self.nc = nc
        self.ctx = ExitStack()
        self.tc_cm = tile.TileContext(nc)

    def __enter__(self):
        self.tc = self.tc_cm.__enter__()
        return self

    def __exit__(self, *a):
        self.ctx.close()
        return self.tc_cm.__exit__(*a)

    _round_i = 0

    def round_copy(self, out, in_):
        nc = self.nc
        nc.vector.tensor_copy(out, in_)

    def build(self, xT_d, wqkT_d, wvT_d, bqk_d, bv_d, wo_d, mask_d, y_d):
        nc, tc, ctx = self.nc, self.tc, self.ctx
        pool = lambda name, bufs, **kw: ctx.enter_context(
            tc.tile_pool(name=name, bufs=bufs, **kw)
        )

        const_p = pool("const", 1)
        wstage_p = pool("wstage", 2)
        xs_p = pool("xs", 1)
        xr_p = pool("xr", 2)
        qkt_p = pool("qkt", 1)
        vones_p = pool("vones", 1)
        attn_p = pool("attn", 3)
        pair_p = pool("pair", 1)
        small_p = pool("small", 2)
        y_p = pool("y", 3)
        # PSUM: ps (2 banks x 2 bufs) + po (1 bank x 2 bufs x 2 tags) = 8
        ps_p = pool("ps", 2, space="PSUM")
        po_p = pool("po", 2, space="PSUM")
        p1_p = ps_p  # qkv/outproj psums share the ps slots

        # ---- constants / weights ----
        bqk_sb = const_p.tile([128, 4], dt.float32, tag="bqk")
        nc.sync.dma_start(bqk_sb[:], bqk_d[:].rearrange("(f p) -> p f", p=128))
        bv_sb = const_p.tile([128, 2], dt.float32, tag="bv")
        nc.sync.dma_start(bv_sb[:], bv_d[:].rearrange("(f p) -> p f", p=128))
        ones_sb = const_p.tile([128, 1, 1], dt.float32, tag="ones")
        nc.vector.memset(ones_sb[:], 1.0)
        mask_sb = const_p.tile([128, 128], dt.float32, tag="mask")
        nc.sync.dma_start(mask_sb[:], mask_d[:])


        wqk_r = []
        for ke in range(KE):
            st = wstage_p.tile([128, FQK], dt.float32, tag="wst")
            nc.sync.dma_start(st[:], wqkT_d[128 * ke : 128 * (ke + 1), :])
            wr = const_p.tile([128, FQK], dt.float32r, tag=f"wqk{ke}")
            self.round_copy(wr[:], st[:])
            wqk_r.append(wr)
        wv_r = []
        for ke in range(KE):
            st = wstage_p.tile([128, FV], dt.float32, tag="wst2")
            nc.sync.dma_start(st[:], wvT_d[128 * ke : 128 * (ke + 1), :])
            wr = const_p.tile([128, FV], dt.float32r, tag=f"wv{ke}")
            self.round_copy(wr[:], st[:])
            wv_r.append(wr)
        wo_r = []
        for kt in range(2):
            st = wstage_p.tile([128, E], dt.float32, tag="wst3")
            nc.sync.dma_start(st[:], wo_d[128 * kt : 128 * (kt + 1), :])
            wr = const_p.tile([128, E], dt.float32r, tag=f"wo{kt}")
            self.round_copy(wr[:], st[:])
            wo_r.append(wr)

        # ---- persistent activations ----
        # qkt tiles: 0: q heads 0,1 | 1: q heads 2,3 | 2: k heads 0,1 | 3: k heads 2,3
        qkt = [qkt_p.tile([128, S], dt.float32r, tag=f"qkt{f}", name=f"qkt{f}") for f in range(4)]
        # vones[t]: [v h0 |1| v h1 |1| v h2 |1| v h3 |1] for token tile t
        vones = [vones_p.tile([128, 4 * 65], dt.float32r, tag=f"v{t}", name=f"v{t}") for t in range(NT)]
        # pair tiles: final normalized attn output, [head dims x 2, S]
        pairt = [pair_p.tile([128, S], dt.float32r, tag=f"pair{hp}", name=f"pair{hp}") for hp in range(2)]

        for c in range(NCH):
            with nc.named_scope(f"qkv{c}"):
                self.qkv_chunk(c, xT_d, wqk_r, wv_r, bqk_sb, ones_sb,
                               xs_p, xr_p, p1_p, qkt, vones)
            for hp in range(2):
                with nc.named_scope(f"attn{c}_{hp}"):
                    self.attention(hp, c, qkt, vones, ps_p, po_p, attn_p,
                                   small_p, pairt, bv_sb, mask_sb)
            with nc.named_scope(f"oproj{c}"):
                self.out_proj(c, pairt, wo_r, p1_p, y_p, y_d)

    # ------------------------------------------------------------------
    def qkv_chunk(self, c, xT_d, wqk_r, wv_r, bqk_sb, ones_sb,
                  xs_p, xr_p, p1_p, qkt, vones):
        nc = self.nc
        cs = slice(512 * c, 512 * (c + 1))
        xr = []
        for ke in range(KE):
            xs = xs_p.tile([128, 512], dt.float32, tag=f"xs{ke}")
            nc.sync.dma_start(xs[:], xT_d[128 * ke : 128 * (ke + 1), cs])
            xt = xr_p.tile([128, 512], dt.float32r, tag=f"xr{ke}")
            self.round_copy(xt[:], xs[:])
            xr.append(xt)
        # q/k projection: qkT[f-tile, chunk] = wqkT.T @ xT
        for f in range(4):
            pq = p1_p.tile([128, 1024], dt.float32, tag="ps", name="pq")
            for ke in range(KE):
                nc.tensor.matmul(
                    pq[:, 0:512], wqk_r[ke][:, 128 * f : 128 * (f + 1)], xr[ke][:],
                    start=(ke == 0), stop=(ke == KE - 1),
                )
            nc.vector.tensor_scalar_add(qkt[f][:, cs], pq[:, 0:512], bqk_sb[:, f : f + 1])
        # v projection, natural layout [tok, vfeat] (no bias: folded post-softmax)
        for t4 in range(4):
            t = 4 * c + t4
            pv = p1_p.tile([128, 1024], dt.float32, tag="ps", name="pv")
            for ke in range(KE):
                nc.tensor.matmul(
                    pv[:, 0:FV],
                    xr[ke][:, 128 * t4 : 128 * (t4 + 1)], wv_r[ke][:],
                    start=(ke == 0), stop=(ke == KE - 1),
                )
            vt = vones[t]
            v3 = vt[:].rearrange("p (g d) -> p g d", d=65)
            nc.vector.tensor_copy(
                v3[:, :, 0:64],
                pv[:, 0:FV].rearrange("p (g d) -> p g d", d=64),
            )
            nc.vector.tensor_copy(v3[:, :, 64:65], ones_sb[:].to_broadcast((128, 4, 1)))

    # ------------------------------------------------------------------
    def attention(self, hp, c, qkt, vones, ps_p, po_p, attn_p, small_p,
                  pairt, bv_sb, mask_sb):
        nc = self.nc
        nj = 4 * c + 4
        po = [po_p.tile([65, 512], dt.float32, tag=f"po{i}", name=f"po{i}") for i in range(2)]

        def emit_pv(j, off, at):
            for h_idx in range(2):
                slot = 2 * hp + h_idx
                nc.tensor.matmul(
                    po[h_idx][:, off:512],
                    vones[j][:, 65 * slot : 65 * slot + 65],
                    at[:, 512 * h_idx + off : 512 * (h_idx + 1)],
                    start=(j == 0), stop=(j == nj - 1),
                    skip_group_check=True,
                )

        prev = None
        for j in range(nj):
            ps = ps_p.tile([128, 1024], dt.float32, tag="ps", name="ps")
            at = attn_p.tile([128, 1024], dt.float32r, tag="attn", name="at")
            m = j - 4 * c
            off = 128 * m if 1 <= m <= 3 else 0
            off_mm = off if m in (1, 2) else 0
            for h_idx in range(2):
                r0 = 64 * h_idx
                nc.tensor.matmul(
                    ps[:, 512 * h_idx + off_mm : 512 * (h_idx + 1)],
                    qkt[2 + hp][r0 : r0 + 64, 128 * j : 128 * (j + 1)],
                    qkt[hp][r0 : r0 + 64, 512 * c + off_mm : 512 * (c + 1)],
                    start=True, stop=True,
                )
            if m >= 0:
                for h_idx in range(2):
                    lo = 512 * h_idx + 128 * m
                    nc.vector.tensor_add(ps[:, lo : lo + 128], ps[:, lo : lo + 128], mask_sb[:])
            if off == 0:
                runs = [(0, 1024)]
            else:
                runs = [(off, 512), (512 + off, 1024)]
            for lo, hi in runs:
                nc.scalar.activation(at[:, lo:hi], ps[:, lo:hi], mybir.ActivationFunctionType.Exp)
            if prev is not None:
                emit_pv(*prev)
            prev = (j, off, at)
        emit_pv(*prev)
        # rollout: normalize by denominator (row 64) and add v bias
        rpa = small_p.tile([1, 1, 512], dt.float32, tag="recipa")
        nc.vector.reciprocal(rpa[0:1, 0, :], po[0][64:65, :])
        rpb = small_p.tile([1, 1, 512], dt.float32, tag="recipb")
        nc.vector.reciprocal(rpb[0:1, 0, :], po[1][64:65, :])
        bc = small_p.tile([128, 512], dt.float32, tag="bcast")
        nc.gpsimd.dma_start(bc[0:64, :], rpa[:].to_broadcast((1, 64, 512)))
        nc.gpsimd.dma_start(bc[64:128, :], rpb[:].to_broadcast((1, 64, 512)))
        tmp = small_p.tile([128, 512], dt.float32, tag="tmp")
        nc.vector.tensor_mul(tmp[0:64, :], po[0][0:64, :], bc[0:64, :])
        nc.vector.tensor_mul(tmp[64:128, :], po[1][0:64, :], bc[64:128, :])
        nc.vector.tensor_scalar_add(
            pairt[hp][:, 512 * c : 512 * (c + 1)], tmp[:], bv_sb[:, hp : hp + 1]
        )

    # ------------------------------------------------------------------
    def out_proj(self, c, pairt, wo_r, p1_p, y_p, y_d):
        nc = self.nc
        for t4 in range(4):
            t = 4 * c + t4
            ysb = y_p.tile([128, E], dt.float32, tag="y")
            for o in range(2):
                py = p1_p.tile([128, 1024], dt.float32, tag="ps", name="py")
                for kt in range(2):
                    nc.tensor.matmul(
                        py[:, 0:512],
                        pairt[kt][:, 128 * t : 128 * (t + 1)],
                        wo_r[kt][:, 512 * o : 512 * (o + 1)],
                        start=(kt == 0), stop=(kt == 1),
                    )
                if o == 0:
                    nc.vector.tensor_copy(ysb[:, 0:512], py[:, 0:512])
                else:
                    nc.scalar.activation(
                        ysb[:, 512:1024], py[:, 0:512], mybir.ActivationFunctionType.Copy
                    )
            nc.gpsimd.dma_start(y_d[128 * t : 128 * (t + 1), :], ysb[:])


# ----------------------------------------------------------------------
_PROGRAM = None


def _get_program():
    global _PROGRAM
    if _PROGRAM is None:
        _PROGRAM = _build_program()
    return _PROGRAM


def _make_in_maps(inputs, W_in, b_in, W_out, b_out):
    in_maps = []
    scale = 1.0 / np.sqrt(np.float32(HD))
    kr = np.arange(128)[:, None]
    qc = np.arange(128)[None, :]
    trimask = np.where(qc >= kr, 0.0, -1e30).astype(np.float32)
    for core in range(NC):
        b, g = divmod(core, 4)
        r = slice(256 * g, 256 * (g + 1))
        wq = W_in[0:E][r] * scale
        wk = W_in[E : 2 * E][r]
        wv = W_in[2 * E : 3 * E][r]
        xT = np.ascontiguousarray(inputs[b].T.astype(np.float32))
        wqkT = np.ascontiguousarray(np.concatenate([wq, wk], axis=0).T)
        wvT = np.ascontiguousarray(wv.T)
        bqk = np.concatenate([b_in[0:E][r] * scale, b_in[E : 2 * E][r]])
        bv = np.ascontiguousarray(b_in[2 * E : 3 * E][r])
        wo = np.ascontiguousarray(W_out[:, r].T)
        in_maps.append(
            {
                "xT": xT,
                "wqkT": wqkT.astype(np.float32),
                "wvT": wvT.astype(np.float32),
                "bqk": bqk.astype(np.float32),
                "bv": bv.astype(np.float32),
                "wo": wo.astype(np.float32),
                "trimask": trimask,
            }
        )
    return in_maps


def run_spmd(inputs, W_in, b_in, W_out, b_out, trace=False, **kw):
    nc = _get_program()
    in_maps = _make_in_maps(inputs, W_in, b_in, W_out, b_out)
    bkr = run_bass_kernel_spmd(nc, in_maps, list(range(NC)), trace=trace, **kw)
    parts = [bkr.results[i]["y"] for i in range(NC)]
    out = np.stack(
        [
            parts[0] + parts[1] + parts[2] + parts[3],
            parts[4] + parts[5] + parts[6] + parts[7],
        ]
    )
    out = out + b_out[None, None, :]
    return out.astype(np.float32), bkr


def kernel(inputs, W_in, b_in, W_out, b_out):
    out, _ = run_spmd(
        np.asarray(inputs, dtype=np.float32),
        np.asarray(W_in, dtype=np.float32),
        np.asarray(b_in, dtype=np.float32),
        np.asarray(W_out, dtype=np.float32),
        np.asarray(b_out, dtype=np.float32),
    )
    return out


# revision 18
# speedup vs baseline: 1.0480x; 1.0075x over previous
"""Trainium2 Bass kernel for causal multi-head attention (B=2, S=2048, E=1024, H=16).

Sharding: 8 cores = 2 batches x 4 head-groups (4 heads each).
Each core computes its batch's QKV for its 4 heads, causal attention, and a
partial output projection; host sums the 4 group partials per batch + b_out.

All big matmuls run in float32r (TF32-like, 1 cycle/row at N>=256).
"""
import sys

sys.path.insert(0, "/opt/trn_rl_repo")

from contextlib import ExitStack

import numpy as np

import concourse.bass as bass
import concourse.tile as tile
from concourse import bacc, mybir
from concourse.bass_utils import run_bass_kernel_spmd

dt = mybir.dt

B, S, E, H = 2, 2048, 1024, 16
HD = 64                     # head dim
HPC = 4                     # heads per core
NC = 8                      # cores
KE = E // 128               # 8 contraction k-tiles for projections
NT = S // 128               # 16 token tiles
NCH = S // 512              # 4 token chunks
FQK = 512                   # q+k features per core (4 heads * 64 * 2)
FV = 256                    # v features per core

# engine used for fp32 -> fp32r rounding copies of DMA'd inputs
ROUND_ENGINE = "gpsimd"


def _build_program():
    nc = bacc.Bacc("TRN2", target_bir_lowering=False, debug=False, num_devices=NC)

    xT_d = nc.dram_tensor("xT", [E, S], dt.float32, kind="ExternalInput")
    wqkT_d = nc.dram_tensor("wqkT", [E, FQK], dt.float32, kind="ExternalInput")
    wvT_d = nc.dram_tensor("wvT", [E, FV], dt.float32, kind="ExternalInput")
    bqk_d = nc.dram_tensor("bqk", [FQK], dt.float32, kind="ExternalInput")
    bv_d = nc.dram_tensor("bv", [FV], dt.float32, kind="ExternalInput")
    wo_d = nc.dram_tensor("wo", [FV, E], dt.float32, kind="ExternalInput")
    mask_d = nc.dram_tensor("trimask", [128, 128], dt.float32, kind="ExternalInput")
    y_d = nc.dram_tensor("y", [S, E], dt.float32, kind="ExternalOutput")

    with TileKernel(nc) as tk:
        tk.build(xT_d, wqkT_d, wvT_d, bqk_d, bv_d, wo_d, mask_d, y_d)
    nc.compile()
    return nc


class TileKernel:
    def __init__(self, nc):
        self.nc = nc
        self.ctx = ExitStack()
        self.tc_cm = tile.TileContext(nc)

    def __enter__(self):
        self.tc = self.tc_cm.__enter__()
        return self

    def __exit__(self, *a):
        self.ctx.close()
        return self.tc_cm.__exit__(*a)

    _round_i = 0

    def round_copy(self, out, in_):
        nc = self.nc
        nc.vector.tensor_copy(out, in_)

    def build(self, xT_d, wqkT_d, wvT_d, bqk_d, bv_d, wo_d, mask_d, y_d):
        nc, tc, ctx = self.nc, self.tc, self.ctx
        pool = lambda name, bufs, **kw: ctx.enter_context(
            tc.tile_pool(name=name, bufs=bufs, **kw)
        )

        const_p = pool("const", 1)
        wstage_p = pool("wstage", 2)
        xs_p = pool("xs", 1)
        xr_p = pool("xr", 2)
        qkt_p = pool("qkt", 1)
        vones_p = pool("vones", 1)
        attn_p = pool("attn", 3)
        pair_p = pool("pair", 1)
        small_p = pool("small", 2)
        y_p = pool("y", 3)
        # PSUM: ps (2 banks x 2 bufs) + po (1 bank x 2 bufs x 2 tags) = 8
        ps_p = pool("ps", 2, space="PSUM")
        po_p = pool("po", 2, space="PSUM")
        p1_p = ps_p  # qkv/outproj psums share the ps slots

        # ---- constants / weights ----
        bqk_sb = const_p.tile([128, 4], dt.float32, tag="bqk")
        nc.sync.dma_start(bqk_sb[:], bqk_d[:].rearrange("(f p) -> p f", p=128))
        bv_sb = const_p.tile([128, 2], dt.float32, tag="bv")
        nc.sync.dma_start(bv_sb[:], bv_d[:].rearrange("(f p) -> p f", p=128))
        ones_sb = const_p.tile([128, 1, 1], dt.float32, tag="ones")
        nc.vector.memset(ones_sb[:], 1.0)
        mask_sb = const_p.tile([128, 128], dt.float32, tag="mask")
        nc.sync.dma_start(mask_sb[:], mask_d[:])


        wqk_r = []
        for ke in range(KE):
            st = wstage_p.tile([128, FQK], dt.float32, tag="wst")
            nc.sync.dma_start(st[:], wqkT_d[128 * ke : 128 * (ke + 1), :])
            wr = const_p.tile([128, FQK], dt.float32r, tag=f"wqk{ke}")
            self.round_copy(wr[:], st[:])
            wqk_r.append(wr)
        wv_r = []
        for ke in range(KE):
            st = wstage_p.tile([128, FV], dt.float32, tag="wst2")
            nc.sync.dma_start(st[:], wvT_d[128 * ke : 128 * (ke + 1), :])
            wr = const_p.tile([128, FV], dt.float32r, tag=f"wv{ke}")
            self.round_copy(wr[:], st[:])
            wv_r.append(wr)
        wo_r = []
        for kt in range(2):
            st = wstage_p.tile([128, E], dt.float32, tag="wst3")
            nc.sync.dma_start(st[:], wo_d[128 * kt : 128 * (kt + 1), :])
            wr = const_p.tile([128, E], dt.float32r, tag=f"wo{kt}")
            self.round_copy(wr[:], st[:])
            wo_r.append(wr)

        # ---- persistent activations ----
        # qkt tiles: 0: q heads 0,1 | 1: q heads 2,3 | 2: k heads 0,1 | 3: k heads 2,3
        qkt = [qkt_p.tile([128, S], dt.float32r, tag=f"qkt{f}", name=f"qkt{f}") for f in range(4)]
        # vones[t]: [v h0 |1| v h1 |1| v h2 |1| v h3 |1] for token tile t
        vones = [vones_p.tile([128, 4 * 65], dt.float32r, tag=f"v{t}", name=f"v{t}") for t in range(NT)]
        # pair tiles: final normalized attn output, [head dims x 2, S]
        pairt = [pair_p.tile([128, S], dt.float32r, tag=f"pair{hp}", name=f"pair{hp}") for hp in range(2)]

        for c in range(NCH):
            with nc.named_scope(f"qkv{c}"):
                self.qkv_chunk(c, xT_d, wqk_r, wv_r, bqk_sb, ones_sb,
                               xs_p, xr_p, p1_p, qkt, vones)
            if c > 0:
                with nc.named_scope(f"oproj{c - 1}"):
                    self.out_proj(c - 1, pairt, wo_r, p1_p, y_p, y_d)
            for hp in range(2):
                with nc.named_scope(f"attn{c}_{hp}"):
                    self.attention(hp, c, qkt, vones, ps_p, po_p, attn_p,
                                   small_p, pairt, bv_sb, mask_sb)
        with nc.named_scope("oproj3"):
            self.out_proj(NCH - 1, pairt, wo_r, p1_p, y_p, y_d)

    # ------------------------------------------------------------------
    def qkv_chunk(self, c, xT_d, wqk_r, wv_r, bqk_sb, ones_sb,
                  xs_p, xr_p, p1_p, qkt, vones):
        nc = self.nc
        cs = slice(512 * c, 512 * (c + 1))
        xr = []
        for ke in range(KE):
            xs = xs_p.tile([128, 512], dt.float32, tag=f"xs{ke}")
            nc.sync.dma_start(xs[:], xT_d[128 * ke : 128 * (ke + 1), cs])
            xt = xr_p.tile([128, 512], dt.float32r, tag=f"xr{ke}")
            self.round_copy(xt[:], xs[:])
            xr.append(xt)
        # q/k projection: qkT[f-tile, chunk] = wqkT.T @ xT
        for f in range(4):
            pq = p1_p.tile([128, 1024], dt.float32, tag="ps", name="pq")
            for ke in range(KE):
                nc.tensor.matmul(
                    pq[:, 0:512], wqk_r[ke][:, 128 * f : 128 * (f + 1)], xr[ke][:],
                    start=(ke == 0), stop=(ke == KE - 1),
                )
            nc.vector.tensor_scalar_add(qkt[f][:, cs], pq[:, 0:512], bqk_sb[:, f : f + 1])
        # v projection, natural layout [tok, vfeat] (no bias: folded post-softmax)
        for t4 in range(4):
            t = 4 * c + t4
            pv = p1_p.tile([128, 1024], dt.float32, tag="ps", name="pv")
            for ke in range(KE):
                nc.tensor.matmul(
                    pv[:, 0:FV],
                    xr[ke][:, 128 * t4 : 128 * (t4 + 1)], wv_r[ke][:],
                    start=(ke == 0), stop=(ke == KE - 1),
                )
            vt = vones[t]
            v3 = vt[:].rearrange("p (g d) -> p g d", d=65)
            nc.vector.tensor_copy(
                v3[:, :, 0:64],
                pv[:, 0:FV].rearrange("p (g d) -> p g d", d=64),
            )
            nc.vector.tensor_copy(v3[:, :, 64:65], ones_sb[:].to_broadcast((128, 4, 1)))

    # ------------------------------------------------------------------
    def attention(self, hp, c, qkt, vones, ps_p, po_p, attn_p, small_p,
                  pairt, bv_sb, mask_sb):
        nc = self.nc
        nj = 4 * c + 4
        po = [po_p.tile([65, 512], dt.float32, tag=f"po{i}", name=f"po{i}") for i in range(2)]

        def emit_pv(j, off, at):
            for h_idx in range(2):
                slot = 2 * hp + h_idx
                nc.tensor.matmul(
                    po[h_idx][:, off:512],
                    vones[j][:, 65 * slot : 65 * slot + 65],
                    at[:, 512 * h_idx + off : 512 * (h_idx + 1)],
                    start=(j == 0), stop=(j == nj - 1),
                    skip_group_check=True,
                )

        prev = None
        for j in range(nj):
            ps = ps_p.tile([128, 1024], dt.float32, tag="ps", name="ps")
            at = attn_p.tile([128, 1024], dt.float32r, tag="attn", name="at")
            m = j - 4 * c
            off = 128 * m if 1 <= m <= 3 else 0
            off_mm = off if m in (1, 2) else 0
            for h_idx in range(2):
                r0 = 64 * h_idx
                nc.tensor.matmul(
                    ps[:, 512 * h_idx + off_mm : 512 * (h_idx + 1)],
                    qkt[2 + hp][r0 : r0 + 64, 128 * j : 128 * (j + 1)],
                    qkt[hp][r0 : r0 + 64, 512 * c + off_mm : 512 * (c + 1)],
                    start=True, stop=True,
                )
            if m >= 0:
                for h_idx in range(2):
                    lo = 512 * h_idx + 128 * m
                    nc.vector.tensor_add(ps[:, lo : lo + 128], ps[:, lo : lo + 128], mask_sb[:])
            if off == 0:
                runs = [(0, 1024)]
            else:
                runs = [(off, 512), (512 + off, 1024)]
            for lo, hi in runs:
                nc.scalar.activation(at[:, lo:hi], ps[:, lo:hi], mybir.ActivationFunctionType.Exp)
            if prev is not None:
                emit_pv(*prev)
            prev = (j, off, at)
        emit_pv(*prev)
        # rollout: normalize by denominator (row 64) and add v bias
        rpa = small_p.tile([1, 1, 512], dt.float32, tag="recipa")
        nc.vector.reciprocal(rpa[0:1, 0, :], po[0][64:65, :])
        rpb = small_p.tile([1, 1, 512], dt.float32, tag="recipb")
        nc.vector.reciprocal(rpb[0:1, 0, :], po[1][64:65, :])
        bc = small_p.tile([128, 512], dt.float32, tag="bcast")
        nc.gpsimd.dma_start(bc[0:64, :], rpa[:].to_broadcast((1, 64, 512)))
        nc.gpsimd.dma_start(bc[64:128, :], rpb[:].to_broadcast((1, 64, 512)))
        tmp = small_p.tile([128, 512], dt.float32, tag="tmp")
        nc.vector.tensor_mul(tmp[0:64, :], po[0][0:64, :], bc[0:64, :])
        nc.vector.tensor_mul(tmp[64:128, :], po[1][0:64, :], bc[64:128, :])
        nc.vector.tensor_scalar_add(
            pairt[hp][:, 512 * c : 512 * (c + 1)], tmp[:], bv_sb[:, hp : hp + 1]
        )

    # ------------------------------------------------------------------
    def out_proj(self, c, pairt, wo_r, p1_p, y_p, y_d):
        nc = self.nc
        for t4 in range(4):
            t = 4 * c + t4
            ysb = y_p.tile([128, E], dt.float32, tag="y")
            for o in range(2):
                py = p1_p.tile([128, 1024], dt.float32, tag="ps", name="py")
                for kt in range(2):
                    nc.tensor.matmul(
                        py[:, 0:512],
                        pairt[kt][:, 128 * t : 128 * (t + 1)],
                        wo_r[kt][:, 512 * o : 512 * (o + 1)],
                        start=(kt == 0), stop=(kt == 1),
                    )
                if o == 0:
                    nc.vector.tensor_copy(ysb[:, 0:512], py[:, 0:512])
                else:
                    nc.scalar.activation(
                        ysb[:, 512:1024], py[:, 0:512], mybir.ActivationFunctionType.Copy
                    )
            nc.gpsimd.dma_start(y_d[128 * t : 128 * (t + 1), :], ysb[:])


# ----------------------------------------------------------------------
_PROGRAM = None


def _get_program():
    global _PROGRAM
    if _PROGRAM is None:
        _PROGRAM = _build_program()
    return _PROGRAM


def _make_in_maps(inputs, W_in, b_in, W_out, b_out):
    in_maps = []
    scale = 1.0 / np.sqrt(np.float32(HD))
    kr = np.arange(128)[:, None]
    qc = np.arange(128)[None, :]
    trimask = np.where(qc >= kr, 0.0, -1e30).astype(np.float32)
    for core in range(NC):
        b, g = divmod(core, 4)
        r = slice(256 * g, 256 * (g + 1))
        wq = W_in[0:E][r] * scale
        wk = W_in[E : 2 * E][r]
        wv = W_in[2 * E : 3 * E][r]
        xT = np.ascontiguousarray(inputs[b].T.astype(np.float32))
        wqkT = np.ascontiguousarray(np.concatenate([wq, wk], axis=0).T)
        wvT = np.ascontiguousarray(wv.T)
        bqk = np.concatenate([b_in[0:E][r] * scale, b_in[E : 2 * E][r]])
        bv = np.ascontiguousarray(b_in[2 * E : 3 * E][r])
        wo = np.ascontiguousarray(W_out[:, r].T)
        in_maps.append(
            {
                "xT": xT,
                "wqkT": wqkT.astype(np.float32),
                "wvT": wvT.astype(np.float32),
                "bqk": bqk.astype(np.float32),
                "bv": bv.astype(np.float32),
                "wo": wo.astype(np.float32),
                "trimask": trimask,
            }
        )
    return in_maps


def run_spmd(inputs, W_in, b_in, W_out, b_out, trace=False, **kw):
    nc = _get_program()
    in_maps = _make_in_maps(inputs, W_in, b_in, W_out, b_out)
    bkr = run_bass_kernel_spmd(nc, in_maps, list(range(NC)), trace=trace, **kw)
    parts = [bkr.results[i]["y"] for i in range(NC)]
    out = np.stack(
        [
            parts[0] + parts[1] + parts[2] + parts[3],
            parts[4] + parts[5] + parts[6] + parts[7],
        ]
    )
    out = out + b_out[None, None, :]
    return out.astype(np.float32), bkr


def kernel(inputs, W_in, b_in, W_out, b_out):
    out, _ = run_spmd(
        np.asarray(inputs, dtype=np.float32),
        np.asarray(W_in, dtype=np.float32),
        np.asarray(b_in, dtype=np.float32),
        np.asarray(W_out, dtype=np.float32),
        np.asarray(b_out, dtype=np.float32),
    )
    return out


# revision 19
# speedup vs baseline: 1.0792x; 1.0298x over previous
"""Trainium2 Bass kernel for causal multi-head attention (B=2, S=2048, E=1024, H=16).

Sharding: 8 cores = 2 batches x 4 head-groups (4 heads each).
Each core computes its batch's QKV for its 4 heads, causal attention, and a
partial output projection; host sums the 4 group partials per batch + b_out.

All big matmuls run in float32r (TF32-like, 1 cycle/row at N>=256).
"""
import sys

sys.path.insert(0, "/opt/trn_rl_repo")

from contextlib import ExitStack

import numpy as np

import concourse.bass as bass
import concourse.tile as tile
from concourse import bacc, mybir
from concourse.bass_utils import run_bass_kernel_spmd

dt = mybir.dt

B, S, E, H = 2, 2048, 1024, 16
HD = 64                     # head dim
HPC = 4                     # heads per core
NC = 8                      # cores
KE = E // 128               # 8 contraction k-tiles for projections
NT = S // 128               # 16 token tiles
NCH = S // 512              # 4 token chunks
FQK = 512                   # q+k features per core (4 heads * 64 * 2)
FV = 256                    # v features per core

# engine used for fp32 -> fp32r rounding copies of DMA'd inputs
ROUND_ENGINE = "gpsimd"


def _build_program():
    nc = bacc.Bacc("TRN2", target_bir_lowering=False, debug=False, num_devices=NC)

    xT_d = nc.dram_tensor("xT", [E, S], dt.float32, kind="ExternalInput")
    wqkT_d = nc.dram_tensor("wqkT", [E, FQK], dt.float32, kind="ExternalInput")
    wvT_d = nc.dram_tensor("wvT", [E, FV], dt.float32, kind="ExternalInput")
    bqk_d = nc.dram_tensor("bqk", [FQK], dt.float32, kind="ExternalInput")
    bv_d = nc.dram_tensor("bv", [FV], dt.float32, kind="ExternalInput")
    wo_d = nc.dram_tensor("wo", [FV, E], dt.float32, kind="ExternalInput")
    mask_d = nc.dram_tensor("trimask", [128, 128], dt.float32, kind="ExternalInput")
    y_d = nc.dram_tensor("y", [S, E], dt.float32, kind="ExternalOutput")

    with TileKernel(nc) as tk:
        tk.build(xT_d, wqkT_d, wvT_d, bqk_d, bv_d, wo_d, mask_d, y_d)
    nc.compile()
    return nc


class TileKernel:
    def __init__(self, nc):
        self.nc = nc
        self.ctx = ExitStack()
        self.tc_cm = tile.TileContext(nc)

    def __enter__(self):
        self.tc = self.tc_cm.__enter__()
        return self

    def __exit__(self, *a):
        self.ctx.close()
        return self.tc_cm.__exit__(*a)

    _round_i = 0

    def round_copy(self, out, in_):
        nc = self.nc
        nc.scalar.activation(out, in_, mybir.ActivationFunctionType.Copy)

    def build(self, xT_d, wqkT_d, wvT_d, bqk_d, bv_d, wo_d, mask_d, y_d):
        nc, tc, ctx = self.nc, self.tc, self.ctx
        pool = lambda name, bufs, **kw: ctx.enter_context(
            tc.tile_pool(name=name, bufs=bufs, **kw)
        )

        const_p = pool("const", 1)
        wstage_p = pool("wstage", 2)
        xs_p = pool("xs", 1)
        xr_p = pool("xr", 2)
        qkt_p = pool("qkt", 1)
        vones_p = pool("vones", 1)
        attn_p = pool("attn", 3)
        pair_p = pool("pair", 1)
        small_p = pool("small", 2)
        y_p = pool("y", 3)
        # PSUM: ps (2 banks x 2 bufs) + po (1 bank x 2 bufs x 2 tags) = 8
        ps_p = pool("ps", 2, space="PSUM")
        po_p = pool("po", 2, space="PSUM")
        p1_p = ps_p  # qkv/outproj psums share the ps slots

        # ---- constants / weights ----
        bqk_sb = const_p.tile([128, 4], dt.float32, tag="bqk")
        nc.sync.dma_start(bqk_sb[:], bqk_d[:].rearrange("(f p) -> p f", p=128))
        bv_sb = const_p.tile([128, 2], dt.float32, tag="bv")
        nc.sync.dma_start(bv_sb[:], bv_d[:].rearrange("(f p) -> p f", p=128))
        ones_sb = const_p.tile([128, 1, 1], dt.float32, tag="ones")
        nc.vector.memset(ones_sb[:], 1.0)
        mask_sb = const_p.tile([128, 128], dt.float32, tag="mask")
        nc.sync.dma_start(mask_sb[:], mask_d[:])


        wqk_r = []
        for ke in range(KE):
            st = wstage_p.tile([128, FQK], dt.float32, tag="wst")
            nc.sync.dma_start(st[:], wqkT_d[128 * ke : 128 * (ke + 1), :])
            wr = const_p.tile([128, FQK], dt.float32r, tag=f"wqk{ke}")
            self.round_copy(wr[:], st[:])
            wqk_r.append(wr)
        wv_r = []
        for ke in range(KE):
            st = wstage_p.tile([128, FV], dt.float32, tag="wst2")
            nc.sync.dma_start(st[:], wvT_d[128 * ke : 128 * (ke + 1), :])
            wr = const_p.tile([128, FV], dt.float32r, tag=f"wv{ke}")
            self.round_copy(wr[:], st[:])
            wv_r.append(wr)
        wo_r = []
        for kt in range(2):
            st = wstage_p.tile([128, E], dt.float32, tag="wst3")
            nc.sync.dma_start(st[:], wo_d[128 * kt : 128 * (kt + 1), :])
            wr = const_p.tile([128, E], dt.float32r, tag=f"wo{kt}")
            self.round_copy(wr[:], st[:])
            wo_r.append(wr)

        # ---- persistent activations ----
        # qkt tiles: 0: q heads 0,1 | 1: q heads 2,3 | 2: k heads 0,1 | 3: k heads 2,3
        qkt = [qkt_p.tile([128, S], dt.float32r, tag=f"qkt{f}", name=f"qkt{f}") for f in range(4)]
        # vones[t]: [v h0 |1| v h1 |1| v h2 |1| v h3 |1] for token tile t
        vones = [vones_p.tile([128, 4 * 65], dt.float32r, tag=f"v{t}", name=f"v{t}") for t in range(NT)]
        # pair tiles: final normalized attn output, [head dims x 2, S]
        pairt = [pair_p.tile([128, S], dt.float32r, tag=f"pair{hp}", name=f"pair{hp}") for hp in range(2)]

        for c in range(NCH):
            with nc.named_scope(f"qkv{c}"):
                self.qkv_chunk(c, xT_d, wqk_r, wv_r, bqk_sb, ones_sb,
                               xs_p, xr_p, p1_p, qkt, vones)
            if c > 0:
                with nc.named_scope(f"oproj{c - 1}"):
                    self.out_proj(c - 1, pairt, wo_r, p1_p, y_p, y_d)
            for hp in range(2):
                with nc.named_scope(f"attn{c}_{hp}"):
                    self.attention(hp, c, qkt, vones, ps_p, po_p, attn_p,
                                   small_p, pairt, bv_sb, mask_sb)
        with nc.named_scope("oproj3"):
            self.out_proj(NCH - 1, pairt, wo_r, p1_p, y_p, y_d)

    # ------------------------------------------------------------------
    def qkv_chunk(self, c, xT_d, wqk_r, wv_r, bqk_sb, ones_sb,
                  xs_p, xr_p, p1_p, qkt, vones):
        nc = self.nc
        cs = slice(512 * c, 512 * (c + 1))
        xr = []
        for ke in range(KE):
            xs = xs_p.tile([128, 512], dt.float32, tag=f"xs{ke}")
            nc.sync.dma_start(xs[:], xT_d[128 * ke : 128 * (ke + 1), cs])
            xt = xr_p.tile([128, 512], dt.float32r, tag=f"xr{ke}")
            self.round_copy(xt[:], xs[:])
            xr.append(xt)
        # q/k projection: qkT[f-tile, chunk] = wqkT.T @ xT
        for f in range(4):
            pq = p1_p.tile([128, 1024], dt.float32, tag="ps", name="pq")
            for ke in range(KE):
                nc.tensor.matmul(
                    pq[:, 0:512], wqk_r[ke][:, 128 * f : 128 * (f + 1)], xr[ke][:],
                    start=(ke == 0), stop=(ke == KE - 1),
                )
            nc.vector.tensor_scalar_add(qkt[f][:, cs], pq[:, 0:512], bqk_sb[:, f : f + 1])
        # v projection, natural layout [tok, vfeat] (no bias: folded post-softmax)
        for t4 in range(4):
            t = 4 * c + t4
            pv = p1_p.tile([128, 1024], dt.float32, tag="ps", name="pv")
            for ke in range(KE):
                nc.tensor.matmul(
                    pv[:, 0:FV],
                    xr[ke][:, 128 * t4 : 128 * (t4 + 1)], wv_r[ke][:],
                    start=(ke == 0), stop=(ke == KE - 1),
                )
            vt = vones[t]
            v3 = vt[:].rearrange("p (g d) -> p g d", d=65)
            nc.vector.tensor_copy(
                v3[:, :, 0:64],
                pv[:, 0:FV].rearrange("p (g d) -> p g d", d=64),
            )
            nc.vector.tensor_copy(v3[:, :, 64:65], ones_sb[:].to_broadcast((128, 4, 1)))

    # ------------------------------------------------------------------
    def attention(self, hp, c, qkt, vones, ps_p, po_p, attn_p, small_p,
                  pairt, bv_sb, mask_sb):
        nc = self.nc
        nj = 4 * c + 4
        po = [po_p.tile([65, 512], dt.float32, tag=f"po{i}", name=f"po{i}") for i in range(2)]

        def emit_pv(j, off, at):
            for h_idx in range(2):
                slot = 2 * hp + h_idx
                nc.tensor.matmul(
                    po[h_idx][:, off:512],
                    vones[j][:, 65 * slot : 65 * slot + 65],
                    at[:, 512 * h_idx + off : 512 * (h_idx + 1)],
                    start=(j == 0), stop=(j == nj - 1),
                    skip_group_check=True,
                )

        prev = None
        for j in range(nj):
            ps = ps_p.tile([128, 1024], dt.float32, tag="ps", name="ps")
            at = attn_p.tile([128, 1024], dt.float32r, tag="attn", name="at")
            m = j - 4 * c
            off = 128 * m if 1 <= m <= 3 else 0
            off_mm = off if m in (1, 2) else 0
            for h_idx in range(2):
                r0 = 64 * h_idx
                nc.tensor.matmul(
                    ps[:, 512 * h_idx + off_mm : 512 * (h_idx + 1)],
                    qkt[2 + hp][r0 : r0 + 64, 128 * j : 128 * (j + 1)],
                    qkt[hp][r0 : r0 + 64, 512 * c + off_mm : 512 * (c + 1)],
                    start=True, stop=True,
                )
            if m >= 0:
                for h_idx in range(2):
                    lo = 512 * h_idx + 128 * m
                    nc.vector.tensor_add(ps[:, lo : lo + 128], ps[:, lo : lo + 128], mask_sb[:])
            if off == 0:
                runs = [(0, 1024)]
            else:
                runs = [(off, 512), (512 + off, 1024)]
            for lo, hi in runs:
                nc.scalar.activation(at[:, lo:hi], ps[:, lo:hi], mybir.ActivationFunctionType.Exp)
            if prev is not None:
                emit_pv(*prev)
            prev = (j, off, at)
        emit_pv(*prev)
        # rollout: normalize by denominator (row 64) and add v bias
        rpa = small_p.tile([1, 1, 512], dt.float32, tag="recipa")
        nc.vector.reciprocal(rpa[0:1, 0, :], po[0][64:65, :])
        rpb = small_p.tile([1, 1, 512], dt.float32, tag="recipb")
        nc.vector.reciprocal(rpb[0:1, 0, :], po[1][64:65, :])
        bc = small_p.tile([128, 512], dt.float32, tag="bcast")
        nc.sync.dma_start(bc[0:64, :], rpa[:].to_broadcast((1, 64, 512)))
        nc.sync.dma_start(bc[64:128, :], rpb[:].to_broadcast((1, 64, 512)))
        tmp = small_p.tile([128, 512], dt.float32, tag="tmp")
        nc.vector.tensor_mul(tmp[0:64, :], po[0][0:64, :], bc[0:64, :])
        nc.vector.tensor_mul(tmp[64:128, :], po[1][0:64, :], bc[64:128, :])
        nc.vector.tensor_scalar_add(
            pairt[hp][:, 512 * c : 512 * (c + 1)], tmp[:], bv_sb[:, hp : hp + 1]
        )

    # ------------------------------------------------------------------
    def out_proj(self, c, pairt, wo_r, p1_p, y_p, y_d):
        nc = self.nc
        for t4 in range(4):
            t = 4 * c + t4
            ysb = y_p.tile([128, E], dt.float32, tag="y")
            for o in range(2):
                py = p1_p.tile([128, 1024], dt.float32, tag="ps", name="py")
                for kt in range(2):
                    nc.tensor.matmul(
                        py[:, 0:512],
                        pairt[kt][:, 128 * t : 128 * (t + 1)],
                        wo_r[kt][:, 512 * o : 512 * (o + 1)],
                        start=(kt == 0), stop=(kt == 1),
                    )
                if o == 0:
                    nc.vector.tensor_copy(ysb[:, 0:512], py[:, 0:512])
                else:
                    nc.scalar.activation(
                        ysb[:, 512:1024], py[:, 0:512], mybir.ActivationFunctionType.Copy
                    )
            nc.gpsimd.dma_start(y_d[128 * t : 128 * (t + 1), :], ysb[:])


# ----------------------------------------------------------------------
_PROGRAM = None


def _get_program():
    global _PROGRAM
    if _PROGRAM is None:
        _PROGRAM = _build_program()
    return _PROGRAM


def _make_in_maps(inputs, W_in, b_in, W_out, b_out):
    in_maps = []
    scale = 1.0 / np.sqrt(np.float32(HD))
    kr = np.arange(128)[:, None]
    qc = np.arange(128)[None, :]
    trimask = np.where(qc >= kr, 0.0, -1e30).astype(np.float32)
    for core in range(NC):
        b, g = divmod(core, 4)
        r = slice(256 * g, 256 * (g + 1))
        wq = W_in[0:E][r] * scale
        wk = W_in[E : 2 * E][r]
        wv = W_in[2 * E : 3 * E][r]
        xT = np.ascontiguousarray(inputs[b].T.astype(np.float32))
        wqkT = np.ascontiguousarray(np.concatenate([wq, wk], axis=0).T)
        wvT = np.ascontiguousarray(wv.T)
        bqk = np.concatenate([b_in[0:E][r] * scale, b_in[E : 2 * E][r]])
        bv = np.ascontiguousarray(b_in[2 * E : 3 * E][r])
        wo = np.ascontiguousarray(W_out[:, r].T)
        in_maps.append(
            {
                "xT": xT,
                "wqkT": wqkT.astype(np.float32),
                "wvT": wvT.astype(np.float32),
                "bqk": bqk.astype(np.float32),
                "bv": bv.astype(np.float32),
                "wo": wo.astype(np.float32),
                "trimask": trimask,
            }
        )
    return in_maps


def run_spmd(inputs, W_in, b_in, W_out, b_out, trace=False, **kw):
    nc = _get_program()
    in_maps = _make_in_maps(inputs, W_in, b_in, W_out, b_out)
    bkr = run_bass_kernel_spmd(nc, in_maps, list(range(NC)), trace=trace, **kw)
    parts = [bkr.results[i]["y"] for i in range(NC)]
    out = np.stack(
        [
            parts[0] + parts[1] + parts[2] + parts[3],
            parts[4] + parts[5] + parts[6] + parts[7],
        ]
    )
    out = out + b_out[None, None, :]
    return out.astype(np.float32), bkr


def kernel(inputs, W_in, b_in, W_out, b_out):
    out, _ = run_spmd(
        np.asarray(inputs, dtype=np.float32),
        np.asarray(W_in, dtype=np.float32),
        np.asarray(b_in, dtype=np.float32),
        np.asarray(W_out, dtype=np.float32),
        np.asarray(b_out, dtype=np.float32),
    )
    return out
